# revision 2
# baseline (speedup 1.0000x reference)
"""Fused single-launch Trainium2 Bass kernel for nn_BidirectionalGlobalCluster.

All three phases (L1: conv/pool/LN/projections; A: attention + anchor
routing/scatter; B: upsample + point routing + output projections) run in
ONE SPMD program over 8 cores, with two AllToAlls for the cross-core
reshards.  Chunks are packed per-RECEIVER so the program is core-id
independent:

  core = img*2 + half  (img in 0..3, half in 0..1)
  exchange 1: sender (img,half) puts head-fc rows (32 ch of dq|dv|ancq|ancv)
              into chunk fc -> A-core fc gets its heads from all 8 cores.
  A-core c handles pairs (n=pr, fc=c), pr in {0,1}.
  exchange 2: A-core windows its msg output per-receiver (22-row bilinear
              source window depends on receiver half) and ships msg+agg to
              chunk r=(img,half).

Host side: one cached jit(shard_map(bass_exec)) callable; donated output
buffers are created device-side (no host->device zeros upload).
"""

import os
import sys
import hashlib
import numpy as np
from concurrent.futures import ThreadPoolExecutor

for _p in ("/opt/trn_rl_repo", "/root/.axon_site/_ro/trn_rl_repo"):
    if os.path.isdir(_p) and _p not in sys.path:
        sys.path.append(_p)

import concourse.bass as bass
import concourse.bacc as bacc
import concourse.mybir as mybir
import concourse.tile as tile

F32 = mybir.dt.float32
F16 = mybir.dt.float16
F32R = mybir.dt.float32r
AFT = mybir.ActivationFunctionType
ALU = mybir.AluOpType
AXX = mybir.AxisListType.X

# hyperparameters (hardcoded per contract)
N_IMG, C, H, W = 4, 256, 160, 160
HID, FC, R, AS = 256, 8, 4, 8
SC = HID // FC            # 32
S = AS * AS               # 64
SH = H // R               # 40
L = SH * SH               # 1600
SCALE = float(C // FC) ** 0.5
LN_EPS = 1e-5
NORM_EPS = 1e-12
NCORE = 8
HH = H // 2               # 80 rows per half
PIX = HH * W              # 12800 pixels per half
LHALF = L // 2            # 800 ds-pixels per half

# exchange-1 layout: chunk fc = rows [32fc:32fc+32] of [dq|dv|ancq|ancv]
X1W = LHALF + LHALF + 32 + 32          # 1664 cols
# exchange-2 (msg): chunk r = 32 rows x MW (the receiver-half 22-row window)
# exchange-3 (agg): chunk r = 4096 elems = the (64,64) agg block, contiguous
RN_WIN = 22
MW = RN_WIN * SH                        # 880
CH2 = 32 * MW                           # 28160 elems per msg chunk
CH3 = 64 * 64                           # 4096 elems per agg chunk


def _ceil(a, b):
    return (a + b - 1) // b


def _newton_recip(nc, tmp_pool, r_ap, d_ap, shape):
    t = tmp_pool.tile(list(shape), F32, tag="newt")
    nc.vector.tensor_tensor(t[:], d_ap, r_ap, ALU.mult)
    nc.vector.tensor_scalar(t[:], t[:], 2.0, -1.0, ALU.subtract, ALU.mult)
    nc.vector.tensor_tensor(r_ap, r_ap, t[:], ALU.mult)


def _newton_rsqrt(nc, tmp_pool, r_ap, x_ap, shape):
    t = tmp_pool.tile(list(shape), F32, tag="newt", name="newt_t")
    nc.vector.tensor_tensor(t[:], r_ap, r_ap, ALU.mult)
    nc.vector.tensor_tensor(t[:], t[:], x_ap, ALU.mult)
    nc.vector.tensor_scalar(t[:], t[:], -0.5, 1.5, ALU.mult, ALU.add)
    nc.vector.tensor_tensor(r_ap, r_ap, t[:], ALU.mult)


def _inv_norm(nc, pool, ssq_ap, shape, eps=NORM_EPS):
    s = pool.tile(list(shape), F32, tag="invn_s", name="invn_s")
    nc.scalar.activation(s[:], ssq_ap, AFT.Sqrt)
    nc.vector.tensor_scalar(s[:], s[:], float(eps), None, ALU.max)
    r = pool.tile(list(shape), F32, tag="invn_r", name="invn_r")
    nc.vector.reciprocal(r[:], s[:])
    _newton_rsqrt(nc, pool, r[:], ssq_ap, shape)
    return r


def _phase_L1(nc, tc, d, ident, xpT_d, a1i):
    SROW = 20  # conv output rows in this half
    with tc.tile_pool(name="const", bufs=1) as cp, \
         tc.tile_pool(name="xin", bufs=1) as xp_pool, \
         tc.tile_pool(name="work", bufs=2) as wk, \
         tc.tile_pool(name="acc", bufs=1) as accp, \
         tc.tile_pool(name="pm", bufs=2) as pmp, \
         tc.tile_pool(name="persist", bufs=1) as prs, \
         tc.tile_pool(name="small", bufs=2) as smp, \
         tc.tile_pool(name="ps", bufs=2, space="PSUM") as ps, \
         tc.tile_pool(name="ps2", bufs=2, space="PSUM") as ps2:

        epsc = cp.tile([128, 1], F32, tag="epsc", name="epsc")
        nc.vector.memset(epsc[:], LN_EPS)

        xsb = []
        for k in range(2):
            t = xp_pool.tile([128, PIX], F32, tag=f"x{k}", name=f"x{k}")
            nc.sync.dma_start(t[:], d["x"][k * 128:(k + 1) * 128, :])
            xsb.append(t)

        wdq = cp.tile([128, 16 * 2], F32, tag="wdq", name="wdq")
        for k in range(2):
            nc.sync.dma_start(wdq[:, k * 16:(k + 1) * 16], d["wdq"][k * 128:(k + 1) * 128, :])
        bdq = cp.tile([128, 2], F32, tag="bdq", name="bdq")
        for k in range(2):
            nc.sync.dma_start(bdq[:, k:k + 1], d["bdq"][k * 128:(k + 1) * 128, :])
        lnw1 = cp.tile([1, C], F32, tag="lnw1", name="lnw1")
        lnb1 = cp.tile([1, C], F32, tag="lnb1", name="lnb1")
        nc.sync.dma_start(lnw1[:], d["lnw"][:, :])
        nc.sync.dma_start(lnb1[:], d["lnb"][:, :])
        lnw = cp.tile([128, C], F32, tag="lnw", name="lnw")
        lnb = cp.tile([128, C], F32, tag="lnb", name="lnb")
        nc.gpsimd.partition_broadcast(lnw[:], lnw1[:])
        nc.gpsimd.partition_broadcast(lnb[:], lnb1[:])

        wmat = {}
        for nm in ("wqk", "wv", "wpt"):
            tl = []
            for k in range(2):
                t = cp.tile([128, HID], F32, tag=f"{nm}{k}", name=f"{nm}{k}")
                nc.sync.dma_start(t[:], d[nm][k * 128:(k + 1) * 128, :])
                tl.append(t)
            wmat[nm] = tl
        bvec = {}
        for nm in ("bqk", "bv", "bpt"):
            t = cp.tile([128, 2], F32, tag=f"{nm}", name=f"{nm}")
            for k in range(2):
                nc.sync.dma_start(t[:, k:k + 1], d[nm][k * 128:(k + 1) * 128, :])
            bvec[nm] = t

        # ---- xp projection (exact fp32) -> local DRAM scratch ----
        NCH = PIX // 512  # 25
        for m in range(2):
            for nchunk in range(NCH):
                pst = ps.tile([128, 512], F32, tag="l1ps", name="xp_ps")
                for k in range(2):
                    nc.tensor.matmul(
                        pst[:], wmat["wpt"][k][:, m * 128:(m + 1) * 128],
                        xsb[k][:, nchunk * 512:(nchunk + 1) * 512],
                        start=(k == 0), stop=(k == 1))
                ot = wk.tile([128, 512], F32, tag="xp_o", name="xp_o")
                nc.scalar.activation(ot[:], pst[:], AFT.Identity,
                                     bias=bvec["bpt"][:, m:m + 1])
                nc.sync.dma_start(
                    xpT_d[m * 128:(m + 1) * 128, nchunk * 512:(nchunk + 1) * 512], ot[:])

        # ---- depthwise conv (DVE) + maxpool ----
        accq = [accp.tile([128, LHALF], F32, tag=f"accq{k}", name=f"accq{k}") for k in range(2)]
        accv = [accp.tile([128, LHALF], F32, tag=f"accv{k}", name=f"accv{k}") for k in range(2)]
        for k in range(2):
            xoff = xsb[k][:].offset
            for idx, (a, b) in enumerate([(a, b) for a in range(4) for b in range(4)]):
                src = bass.AP(xsb[k].tensor, xoff + a * W + b,
                              [[PIX, 128], [4 * W, SROW], [4, SH]])
                wcol = wdq[:, k * 16 + idx:k * 16 + idx + 1]
                if idx == 0:
                    nc.vector.tensor_scalar(accq[k][:], src, wcol, None, ALU.mult)
                else:
                    nc.vector.scalar_tensor_tensor(
                        accq[k][:], src, wcol, accq[k][:], ALU.mult, ALU.add)
            nc.vector.tensor_scalar(accq[k][:], accq[k][:], bdq[:, k:k + 1], None, ALU.add)
            ptmp = accp.tile([128, SROW * SH * 4], F32, tag=f"ptmp{k}", name=f"ptmp{k}")
            nc.vector.tensor_reduce(
                ptmp[:],
                bass.AP(xsb[k].tensor, xoff,
                        [[PIX, 128], [4 * W, SROW], [4, SH], [W, 4], [1, 4]]),
                AXX, ALU.max)
            nc.vector.tensor_reduce(
                accv[k][:],
                bass.AP(ptmp.tensor, ptmp[:].offset,
                        [[SROW * SH * 4, 128], [SH * 4, SROW], [4, SH], [1, 4]]),
                AXX, ALU.max)

        # ---- LN + projection; results -> a1i (exchange-1 input) ----
        NPM = _ceil(LHALF, 128)  # 7 chunks, last=32
        for path, acc, wnm, bnm, ccol0 in (
                ("q", accq, "wqk", "bqk", 0),
                ("v", accv, "wv", "bv", LHALF)):
            nrmT = [prs.tile([128, LHALF], F32, tag=f"nrmT{path}{k}", name=f"nrmT{path}{k}") for k in range(2)]
            for pc in range(NPM):
                sz = min(128, LHALF - pc * 128)
                pm = pmp.tile([128, C], F32, tag=f"pm{path}", name=f"pm{path}")
                for k in range(2):
                    pt = ps2.tile([128, 128], F32, tag="tp_ps", name="tp_ps")
                    nc.tensor.transpose(
                        pt[0:sz, :], acc[k][:, pc * 128:pc * 128 + sz],
                        ident[:, :])
                    nc.scalar.copy(pm[0:sz, k * 128:(k + 1) * 128], pt[0:sz, 0:128])
                mu = smp.tile([128, 1], F32, tag=f"mu{path}", name=f"mu{path}")
                nc.vector.reduce_sum(mu[0:sz, :], pm[0:sz, :], AXX)
                nc.vector.tensor_scalar(mu[0:sz, :], mu[0:sz, :], 1.0 / C, None, ALU.mult)
                cent = pmp.tile([128, C], F32, tag=f"cent{path}", name=f"cent{path}")
                nc.vector.tensor_scalar(cent[0:sz, :], pm[0:sz, :], mu[0:sz, :], None, ALU.subtract)
                var = smp.tile([128, 1], F32, tag=f"var{path}", name=f"var{path}")
                sq = pmp.tile([128, C], F32, tag=f"sq{path}", name=f"sq{path}")
                nc.scalar.activation(sq[0:sz, :], cent[0:sz, :], AFT.Square,
                                     accum_out=var[0:sz, :])
                nc.vector.tensor_scalar(var[0:sz, :], var[0:sz, :], 1.0 / C, None, ALU.mult)
                vpe = smp.tile([128, 1], F32, tag=f"vpe{path}", name=f"vpe{path}")
                nc.vector.tensor_scalar(vpe[0:sz, :], var[0:sz, :], LN_EPS, None, ALU.add)
                istd = smp.tile([128, 1], F32, tag=f"istd{path}", name=f"istd{path}")
                nc.scalar.activation(istd[0:sz, :], vpe[0:sz, :], AFT.Sqrt)
                irec = smp.tile([128, 1], F32, tag=f"irec{path}", name=f"irec{path}")
                nc.vector.reciprocal(irec[0:sz, :], istd[0:sz, :])
                _newton_rsqrt(nc, smp, irec[0:sz, :], vpe[0:sz, :], (sz, 1))
                nc.vector.tensor_scalar(cent[0:sz, :], cent[0:sz, :], irec[0:sz, :], None, ALU.mult)
                nc.vector.tensor_tensor(cent[0:sz, :], cent[0:sz, :], lnw[0:sz, :], ALU.mult)
                nc.vector.tensor_tensor(cent[0:sz, :], cent[0:sz, :], lnb[0:sz, :], ALU.add)
                for k in range(2):
                    pt = ps2.tile([128, 128], F32, tag="tp_ps", name="tp_ps")
                    nc.tensor.transpose(
                        pt[0:128, 0:sz], cent[0:sz, k * 128:(k + 1) * 128],
                        ident[0:sz, 0:sz])
                    nc.scalar.copy(nrmT[k][:, pc * 128:pc * 128 + sz], pt[0:128, 0:sz])
            projT = [prs.tile([128, LHALF], F32, tag=f"projT{path}{m}", name=f"projT{path}{m}") for m in range(2)]
            for m in range(2):
                for n0 in range(0, LHALF, 512):
                    nsz = min(512, LHALF - n0)
                    pst = ps.tile([128, 512], F32, tag="l1ps", name="proj_ps")
                    for k in range(2):
                        nc.tensor.matmul(
                            pst[:, 0:nsz], wmat[wnm][k][:, m * 128:(m + 1) * 128],
                            nrmT[k][:, n0:n0 + nsz], start=(k == 0), stop=(k == 1))
                    nc.scalar.activation(projT[m][:, n0:n0 + nsz], pst[:, 0:nsz],
                                         AFT.Identity, bias=bvec[bnm][:, m:m + 1])
                nc.sync.dma_start(
                    bass.AP(a1i.tensor, a1i[:].offset + m * 128 * X1W + ccol0,
                            [[X1W, 128], [1, LHALF]]),
                    projT[m][:])
            # anchors: 5x5 block means -> (4, 8) per channel -> a1i anc cols
            acol0 = 2 * LHALF + (0 if path == "q" else 32)
            for m in range(2):
                anc = smp.tile([128, 32], F32, tag=f"anc{path}", name=f"anc{path}")
                for idx, (di, dj) in enumerate([(i, j) for i in range(5) for j in range(5)]):
                    src = bass.AP(projT[m].tensor, projT[m][:].offset + di * SH + dj,
                                  [[LHALF, 128], [5 * SH, 4], [5, 8]])
                    if idx == 0:
                        nc.vector.tensor_copy(anc[:], src)
                    else:
                        nc.vector.tensor_tensor(anc[:], anc[:], src, ALU.add)
                nc.vector.tensor_scalar(anc[:], anc[:], 1.0 / 25.0, None, ALU.mult)
                nc.sync.dma_start(
                    bass.AP(a1i.tensor, a1i[:].offset + m * 128 * X1W + acol0,
                            [[X1W, 128], [1, 32]]),
                    anc[:])


def _phase_A(nc, tc, d, ident, a1o, a2i, a3i):
    NLC = _ceil(L, 128)  # 13 chunks, last 64
    with tc.tile_pool(name="constA", bufs=1) as cp, \
         tc.tile_pool(name="feat", bufs=1) as fp, \
         tc.tile_pool(name="featr", bufs=1) as fr, \
         tc.tile_pool(name="pmf", bufs=1) as pmf, \
         tc.tile_pool(name="stripe", bufs=3) as stp, \
         tc.tile_pool(name="smallA", bufs=2) as smp, \
         tc.tile_pool(name="msg", bufs=2) as msgp, \
         tc.tile_pool(name="ps_sim", bufs=2, space="PSUM") as ps_sim, \
         tc.tile_pool(name="ps_msg", bufs=1, space="PSUM") as ps_msg, \
         tc.tile_pool(name="ps_agg", bufs=1, space="PSUM") as ps_agg, \
         tc.tile_pool(name="ps_smA", bufs=1, space="PSUM") as ps_sm:

        dab1 = cp.tile([1, 2], F32, tag="dab1", name="dab1")
        nc.sync.dma_start(dab1[:], d["dab"][:, :])
        dab = cp.tile([128, 2], F32, tag="dab", name="dab")
        nc.gpsimd.partition_broadcast(dab[:], dab1[:])
        ones13 = cp.tile([128, 13], F32, tag="ones13", name="ones13")
        nc.vector.memset(ones13[:], 1.0)

        a1off = a1o[:].offset

        def x1src(s, col0, w):
            # 32 rows of exchange-1 chunk s, cols [col0:col0+w]
            return bass.AP(a1o.tensor, a1off + 32 * s * X1W + col0,
                           [[X1W, 32], [1, w]])

        # tiles indexed j = pr*2 + im ; source core s = (im*2+pr)*2 + h
        qkT, vT, aqkT, avvT, qkR = [], [], [], [], []
        for j in range(4):
            pr, im = j // 2, j % 2
            t = fp.tile([32, L], F32, tag=f"qkT{j}", name=f"qkT{j}")
            tr = fr.tile([32, L], F32R, tag=f"qkR{j}", name=f"qkR{j}")
            tv = fp.tile([32, L], F32, tag=f"vT{j}", name=f"vT{j}")
            ta = fp.tile([32, S], F32, tag=f"aqkT{j}", name=f"aqkT{j}")
            tav = fp.tile([32, S], F32, tag=f"avvT{j}", name=f"avvT{j}")
            for h in range(2):
                s = (im * 2 + pr) * 2 + h
                nc.sync.dma_start(t[:, h * LHALF:(h + 1) * LHALF], x1src(s, 0, LHALF))
                nc.gpsimd.dma_start(tr[:, h * LHALF:(h + 1) * LHALF], x1src(s, 0, LHALF))
                nc.sync.dma_start(tv[:, h * LHALF:(h + 1) * LHALF], x1src(s, LHALF, LHALF))
                nc.sync.dma_start(ta[:, h * 32:(h + 1) * 32], x1src(s, 2 * LHALF, 32))
                nc.sync.dma_start(tav[:, h * 32:(h + 1) * 32], x1src(s, 2 * LHALF + 32, 32))
            qkT.append(t)
            qkR.append(tr)
            vT.append(tv)
            aqkT.append(ta)
            avvT.append(tav)

        a2off = a2i[:].offset
        a3off = a3i[:].offset

        for pr in range(2):
            gi = lambda img: pr * 2 + img

            # ---- pixel-major copies + [v|1] lhsT tiles per image ----
            v_pm, qk_pm, v1R = [], [], []
            for img in range(2):
                vpm_t = pmf.tile([128, 32 * NLC], F32, tag=f"vpm{pr}{img}", name=f"vpm{pr}{img}")
                qpm_t = pmf.tile([128, 32 * NLC], F32, tag=f"qpm{pr}{img}", name=f"qpm{pr}{img}")
                v1r_t = pmf.tile([128, 33 * NLC], F32R, tag=f"v1r{pr}{img}", name=f"v1r{pr}{img}")
                nc.gpsimd.dma_start(
                    bass.AP(v1r_t.tensor, v1r_t[:].offset + 32,
                            [[33 * NLC, 128], [33, NLC]]),
                    ones13[:])
                for lc in range(NLC):
                    sz = min(128, L - lc * 128)
                    pt = ps_sm.tile([128, 32], F32, tag="sm", name="tpA")
                    nc.tensor.transpose(
                        pt[0:sz, :], vT[gi(img)][0:32, lc * 128:lc * 128 + sz],
                        ident[0:32, 0:32])
                    nc.scalar.copy(vpm_t[0:sz, lc * 32:(lc + 1) * 32], pt[0:sz, :])
                    nc.scalar.copy(v1r_t[0:sz, lc * 33:lc * 33 + 32], pt[0:sz, :])
                    pt2 = ps_sm.tile([128, 32], F32, tag="sm", name="tpA")
                    nc.tensor.transpose(
                        pt2[0:sz, :], qkT[gi(img)][0:32, lc * 128:lc * 128 + sz],
                        ident[0:32, 0:32])
                    nc.scalar.copy(qpm_t[0:sz, lc * 32:(lc + 1) * 32], pt2[0:sz, :])
                v_pm.append(vpm_t)
                qk_pm.append(qpm_t)
                v1R.append(v1r_t)

            # ---- attention ----
            for dr in range(2):
                i_q, i_k = (0, 1) if dr == 0 else (1, 0)
                msgT_ps = ps_msg.tile([33, L], F32, tag="msgT_ps", name="msgT_ps")
                for lc in range(NLC):
                    sz = min(128, L - lc * 128)
                    stripe = stp.tile([128, L], F32R, tag="stripe", name="stripe")
                    for n0 in range(0, L, 512):
                        nsz = min(512, L - n0)
                        pst = ps_sim.tile([128, 512], F32, tag="sim_ps", name="sim_ps")
                        nc.tensor.matmul(
                            pst[0:sz, 0:nsz],
                            qkR[gi(i_q)][0:32, lc * 128:lc * 128 + sz],
                            qkR[gi(i_k)][0:32, n0:n0 + nsz],
                            start=True, stop=True)
                        nc.scalar.activation(stripe[0:sz, n0:n0 + nsz], pst[0:sz, 0:nsz],
                                             AFT.Exp, scale=1.0 / SCALE)
                    for n0 in range(0, L, 512):
                        nsz = min(512, L - n0)
                        nc.tensor.matmul(
                            msgT_ps[:, n0:n0 + nsz],
                            bass.AP(v1R[i_q].tensor, v1R[i_q][:].offset + lc * 33,
                                    [[33 * NLC, sz], [1, 33]]),
                            stripe[0:sz, n0:n0 + nsz],
                            start=(lc == 0), stop=(lc == NLC - 1))
                msgT_sb = msgp.tile([33, L], F32, tag="msgT_sb", name="msgT_sb")
                nc.scalar.copy(msgT_sb[:], msgT_ps[:])
                msgT_out = msgp.tile([32, L], F32, tag="msgT_out", name="msgT_out")
                for lc in range(NLC):
                    sz = min(128, L - lc * 128)
                    pt = ps_sm.tile([128, 33], F32, tag="sm", name="msg_tp")
                    nc.tensor.transpose(
                        pt[0:sz, :], msgT_sb[:, lc * 128:lc * 128 + sz],
                        ident[0:33, 0:33])
                    den = smp.tile([128, 1], F32, tag="den", name="den")
                    nc.vector.reciprocal(den[0:sz, :], pt[0:sz, 32:33])
                    pm = smp.tile([128, 32], F32, tag="msg_pm", name="msg_pm")
                    nc.vector.tensor_scalar(pm[0:sz, :], pt[0:sz, 0:32], den[0:sz, :],
                                            None, ALU.mult)
                    pt2 = ps_sm.tile([32, 128], F32, tag="sm", name="msg_tp2")
                    nc.tensor.transpose(pt2[:, 0:sz], pm[0:sz, :], ident[0:sz, 0:sz])
                    nc.scalar.copy(msgT_out[:, lc * 128:lc * 128 + sz], pt2[:, 0:sz])
                od = 1 - dr
                # ship windowed msg to receivers (img=od*2+pr, h) for h in 0,1
                moff = msgT_out[:].offset
                for h in range(2):
                    r = (od * 2 + pr) * 2 + h
                    dbase = a2off + r * CH2
                    if h == 0:
                        # window j=0 <- grid row 0 ; j=1..21 <- rows 0..20
                        nc.sync.dma_start(
                            bass.AP(a2i.tensor, dbase, [[MW, 32], [1, SH]]),
                            bass.AP(msgT_out.tensor, moff, [[L, 32], [1, SH]]))
                        nc.sync.dma_start(
                            bass.AP(a2i.tensor, dbase + SH,
                                    [[MW, 32], [SH, 21], [1, SH]]),
                            bass.AP(msgT_out.tensor, moff, [[L, 32], [SH, 21], [1, SH]]))
                    else:
                        # window j=0..20 <- rows 19..39 ; j=21 <- row 39
                        nc.sync.dma_start(
                            bass.AP(a2i.tensor, dbase,
                                    [[MW, 32], [SH, 21], [1, SH]]),
                            bass.AP(msgT_out.tensor, moff + 19 * SH,
                                    [[L, 32], [SH, 21], [1, SH]]))
                        nc.sync.dma_start(
                            bass.AP(a2i.tensor, dbase + 21 * SH, [[MW, 32], [1, SH]]),
                            bass.AP(msgT_out.tensor, moff + 39 * SH, [[L, 32], [1, SH]]))

            # ---- routing + scatter aggregation per image/head ----
            for img in range(2):
                apm = smp.tile([64, 32], F32, tag="apm", name="apm")
                avm = smp.tile([64, 32], F32, tag="avm", name="avm")
                pt = ps_sm.tile([64, 128], F32, tag="sm", name="anc_tp")
                nc.tensor.transpose(pt[:, 0:32], aqkT[gi(img)][0:32, :],
                                    ident[0:32, 0:32])
                nc.scalar.copy(apm[:], pt[:, 0:32])
                pt = ps_sm.tile([64, 128], F32, tag="sm", name="anc_tp")
                nc.tensor.transpose(pt[:, 0:32], avvT[gi(img)][0:32, :],
                                    ident[0:32, 0:32])
                nc.scalar.copy(avm[:], pt[:, 0:32])
                ssq = smp.tile([64, 1], F32, tag="assq", name="assq")
                sq = smp.tile([64, 32], F32, tag="asq", name="asq")
                nc.scalar.activation(sq[:], apm[:], AFT.Square, accum_out=ssq[:])
                inv = _inv_norm(nc, smp, ssq[:], (64, 1))
                apn = smp.tile([64, 32], F32, tag="apn", name="apn")
                nc.vector.tensor_scalar(apn[:], apm[:], inv[:], None, ALU.mult)
                apnT_ps = ps_sm.tile([32, 64], F32, tag="sm", name="apnT_ps")
                nc.tensor.transpose(apnT_ps[:], apn[:], ident[0:64, 0:64])
                apnT = smp.tile([32, 64], F32, tag="apnT", name="apnT")
                nc.scalar.copy(apnT[:], apnT_ps[:])

                aggps = ps_agg.tile([64, 65], F32, tag="agg_ps", name="agg_ps")
                for lc in range(NLC):
                    sz = min(128, L - lc * 128)
                    raw = ps_sm.tile([128, 64], F32, tag="sm", name="raw_ps")
                    nc.tensor.matmul(raw[0:sz, :],
                                     qkT[gi(img)][0:32, lc * 128:lc * 128 + sz],
                                     apnT[:], start=True, stop=True)
                    pssq = smp.tile([128, 1], F32, tag="pssq", name="pssq")
                    psq = smp.tile([128, 32], F32, tag="psq", name="psq")
                    qslice = bass.AP(qk_pm[img].tensor, qk_pm[img][:].offset + lc * 32,
                                     [[32 * NLC, sz], [1, 32]])
                    nc.scalar.activation(psq[0:sz, :], qslice, AFT.Square,
                                         accum_out=pssq[0:sz, :])
                    pinv = _inv_norm(nc, smp, pssq[0:sz, :], (sz, 1))
                    sca = smp.tile([128, 1], F32, tag="sca", name="sca")
                    nc.vector.tensor_tensor(sca[0:sz, :], pinv[0:sz, :], dab[0:sz, 0:1],
                                            ALU.mult)
                    mx = smp.tile([128, 1], F32, tag="mx", name="mx")
                    nc.vector.reduce_max(mx[0:sz, :], raw[0:sz, :], AXX)
                    vals = smp.tile([128, 1], F32, tag="vals", name="vals")
                    nc.scalar.activation(vals[0:sz, :], mx[0:sz, :], AFT.Sigmoid,
                                         bias=dab[0:sz, 1:2], scale=sca[0:sz, :])
                    maskW = smp.tile([128, 64], F32, tag="maskW", name="maskW")
                    nc.vector.tensor_scalar(maskW[0:sz, :], raw[0:sz, :], mx[0:sz, :],
                                            vals[0:sz, :], ALU.is_equal, ALU.mult)
                    rhs = smp.tile([128, 65], F32, tag="agg_rhs", name="agg_rhs")
                    nc.vector.tensor_copy(rhs[0:sz, 0:32], qslice)
                    nc.vector.tensor_copy(
                        rhs[0:sz, 32:64],
                        bass.AP(v_pm[img].tensor, v_pm[img][:].offset + lc * 32,
                                [[32 * NLC, sz], [1, 32]]))
                    nc.vector.memset(rhs[0:sz, 64:65], 1.0)
                    nc.tensor.matmul(aggps[:], maskW[0:sz, :], rhs[0:sz, :],
                                     start=(lc == 0), stop=(lc == NLC - 1))
                aggsb = smp.tile([64, 65], F32, tag="aggsb", name="aggsb")
                nc.vector.tensor_copy(aggsb[:, 0:32], apm[:])
                nc.vector.tensor_copy(aggsb[:, 32:64], avm[:])
                nc.vector.memset(aggsb[:, 64:65], 1.0)
                nc.vector.tensor_tensor(aggsb[:], aggsb[:], aggps[:], ALU.add)
                den = smp.tile([64, 1], F32, tag="aden", name="aden")
                nc.vector.reciprocal(den[:], aggsb[:, 64:65])
                _newton_recip(nc, smp, den[:], aggsb[:, 64:65], (64, 1))
                outa = smp.tile([64, 64], F32, tag="outa", name="outa")
                nc.vector.tensor_scalar(outa[:], aggsb[:, 0:64], den[:], None, ALU.mult)
                # ship agg to receivers r = ((1-img)*2+pr)*2 + h
                for h in range(2):
                    r = ((1 - img) * 2 + pr) * 2 + h
                    nc.sync.dma_start(
                        bass.AP(a3i.tensor, a3off + r * CH3,
                                [[64, 64], [1, 64]]),
                        outa[:])


def _upsample_rows():
    rows = []
    for r in range(HH):
        s = (r + 0.5) / 4.0 - 0.5
        a = int(np.floor(s))
        fb = s - a
        rows.append((a + 1, a + 2, 1.0 - fb, fb))
    return rows


def _phase_B(nc, tc, d, ident, a2o, a3o, xpT_d):
    NBLK = 16
    BP = PIX // NBLK          # 800 pixels per block (5 rows)
    BROWS = HH // NBLK        # 5
    rows_tab = _upsample_rows()
    a2off = a2o[:].offset
    a3off = a3o[:].offset

    with tc.tile_pool(name="constB", bufs=1) as cp, \
         tc.tile_pool(name="stage1", bufs=1) as s1, \
         tc.tile_pool(name="hzp", bufs=1) as hzp, \
         tc.tile_pool(name="blk", bufs=2) as blk, \
         tc.tile_pool(name="blk1", bufs=1) as blk1, \
         tc.tile_pool(name="smallB", bufs=2) as smp, \
         tc.tile_pool(name="maskp", bufs=1) as mkp, \
         tc.tile_pool(name="outp", bufs=2) as outp, \
         tc.tile_pool(name="ps1", bufs=2, space="PSUM") as ps1, \
         tc.tile_pool(name="ps_raw", bufs=2, space="PSUM") as ps_raw, \
         tc.tile_pool(name="ps_tp", bufs=2, space="PSUM") as ps_tp, \
         tc.tile_pool(name="ps_dsp", bufs=2, space="PSUM") as ps_dsp:

        ab1 = cp.tile([1, 2], F32, tag="ab1", name="ab1")
        nc.sync.dma_start(ab1[:], d["ab"][:, :])
        ab = cp.tile([128, 2], F32, tag="ab", name="ab")
        nc.gpsimd.partition_broadcast(ab[:], ab1[:])
        bd8r = [cp.tile([64, 8], F32R, tag=f"bd8r{j}", name=f"bd8r{j}") for j in range(4)]
        for j in range(4):
            nc.gpsimd.dma_start(bd8r[j][:], d["bd8"][j * 64:(j + 1) * 64, :])
        bm0 = cp.tile([128, 2], F32, tag="bm0", name="bm0")
        bm1 = cp.tile([128, 2], F32, tag="bm1", name="bm1")
        for k in range(2):
            nc.sync.dma_start(bm0[:, k:k + 1], d["bm0"][k * 128:(k + 1) * 128, :])
            nc.sync.dma_start(bm1[:, k:k + 1], d["bm1"][k * 128:(k + 1) * 128, :])
        wm0R = [cp.tile([128, C], F32R, tag=f"wm0R{k}", name=f"wm0R{k}") for k in range(2)]
        for k in range(2):
            nc.gpsimd.dma_start(wm0R[k][:], d["wm0"][k * 128:(k + 1) * 128, :])
        wm1R = [cp.tile([32, C], F32R, tag=f"wm1R{h}", name=f"wm1R{h}") for h in range(8)]
        avR = [cp.tile([64, 32], F32R, tag=f"avR{h}", name=f"avR{h}") for h in range(8)]
        for h in range(8):
            nc.gpsimd.dma_start(wm1R[h][:], d["wm1"][h * 32:(h + 1) * 32, :])
            nc.gpsimd.dma_start(
                avR[h][:],
                bass.AP(a3o.tensor, a3off + h * CH3 + 32,
                        [[64, 64], [1, 32]]))

        # normalized anchor transposes per head
        apnP = []
        for h in range(8):
            apm_t = smp.tile([64, 32], F32, tag="apm_t", name="apm_t")
            nc.sync.dma_start(
                apm_t[:],
                bass.AP(a3o.tensor, a3off + h * CH3,
                        [[64, 64], [1, 32]]))
            ssq = smp.tile([64, 1], F32, tag="apssq", name="apssq")
            sq = smp.tile([64, 32], F32, tag="apsq", name="apsq")
            nc.scalar.activation(sq[:], apm_t[:], AFT.Square, accum_out=ssq[:])
            inv = _inv_norm(nc, smp, ssq[:], (64, 1))
            apn = smp.tile([64, 32], F32, tag="apn", name="apn")
            nc.vector.tensor_scalar(apn[:], apm_t[:], inv[:], None, ALU.mult)
            pt = ps_tp.tile([32, 64], F32, tag="tp", name="apnT_ps")
            nc.tensor.transpose(pt[:], apn[:], ident[0:64, 0:64])
            at = cp.tile([32, 64], F32, tag=f"apnT{h}", name=f"apnT{h}")
            nc.scalar.copy(at[:], pt[:])
            bp = (h % 2) * 32
            atp = cp.tile([64, 64], F32, tag=f"apnP{h}", name=f"apnP{h}")
            nc.sync.dma_start(atp[bp:bp + 32, :], at[:])
            apnP.append(atp)

        # ---- stage 1: merge msgs (22-row window) + horizontal upsample ----
        mfR = [s1.tile([128, MW], F32R, tag=f"mfR{k}", name=f"mfR{k}") for k in range(2)]
        for k in range(2):
            for fcm in range(4):
                fc = k * 4 + fcm
                nc.gpsimd.dma_start(
                    mfR[k][fcm * 32:(fcm + 1) * 32, :],
                    bass.AP(a2o.tensor, a2off + fc * CH2, [[MW, 32], [1, MW]]))
        attm = [s1.tile([128, MW], F32, tag=f"attm{m}", name=f"attm{m}") for m in range(2)]
        for m in range(2):
            for n0 in range(0, MW, 512):
                nsz = min(512, MW - n0)
                pst = ps1.tile([128, 512], F32, tag="big_ps", name="mrg_ps")
                for k in range(2):
                    nc.tensor.matmul(pst[:, 0:nsz], wm0R[k][:, m * 128:(m + 1) * 128],
                                     mfR[k][:, n0:n0 + nsz], start=(k == 0), stop=(k == 1))
                nc.scalar.activation(attm[m][:, n0:n0 + nsz], pst[:, 0:nsz],
                                     AFT.Identity, bias=bm0[:, m:m + 1])
        hz = [hzp.tile([128, RN_WIN * W], F32, tag=f"hz{m}", name=f"hz{m}") for m in range(2)]
        PHI = [(0.375, -1, 0.625, 0), (0.125, -1, 0.875, 0),
               (0.875, 0, 0.125, 1), (0.625, 0, 0.375, 1)]
        for m in range(2):
            am = attm[m]
            hzm = hz[m]
            for phi, (wa, da, wb, db) in enumerate(PHI):
                t0 = 0 if da >= 0 else 1
                t1 = SH if db <= 0 else SH - 1
                cnt = t1 - t0
                out_ap = bass.AP(hzm.tensor, hzm[:].offset + 4 * t0 + phi,
                                 [[RN_WIN * W, 128], [W, RN_WIN], [4, cnt]])
                src_a = bass.AP(am.tensor, am[:].offset + t0 + da,
                                [[MW, 128], [SH, RN_WIN], [1, cnt]])
                src_b = bass.AP(am.tensor, am[:].offset + t0 + db,
                                [[MW, 128], [SH, RN_WIN], [1, cnt]])
                nc.scalar.mul(out_ap, src_b, wb)
                nc.vector.scalar_tensor_tensor(out_ap, src_a, wa, out_ap, ALU.mult, ALU.add)
                if da < 0:
                    edge_out = bass.AP(hzm.tensor, hzm[:].offset + phi,
                                       [[RN_WIN * W, 128], [W, RN_WIN]])
                    edge_src = bass.AP(am.tensor, am[:].offset + 0, [[MW, 128], [SH, RN_WIN]])
                else:
                    edge_out = bass.AP(hzm.tensor, hzm[:].offset + 4 * (SH - 1) + phi,
                                       [[RN_WIN * W, 128], [W, RN_WIN]])
                    edge_src = bass.AP(am.tensor, am[:].offset + SH - 1,
                                       [[MW, 128], [SH, RN_WIN]])
                nc.scalar.copy(edge_out, edge_src)

        # ---- stage 2: per block ----
        NPC = _ceil(BP, 128)
        for bi in range(NBLK):
            for m in range(2):
                oat = outp.tile([128, BP], F32, tag=f"oat{m}", name=f"oat{m}")
                r0 = bi * BROWS
                for r in range(BROWS):
                    sa, sb, wa, wb = rows_tab[r0 + r]
                    out_ap = bass.AP(oat.tensor, oat[:].offset + r * W, [[BP, 128], [1, W]])
                    sa_ap = bass.AP(hz[m].tensor, hz[m][:].offset + sa * W,
                                    [[RN_WIN * W, 128], [1, W]])
                    sb_ap = bass.AP(hz[m].tensor, hz[m][:].offset + sb * W,
                                    [[RN_WIN * W, 128], [1, W]])
                    nc.scalar.mul(out_ap, sb_ap, wb)
                    nc.vector.scalar_tensor_tensor(out_ap, sa_ap, wa, out_ap,
                                                   ALU.mult, ALU.add)
                oat16 = outp.tile([128, BP], F16, tag=f"oat16{m}", name=f"oat16{m}")
                nc.scalar.copy(oat16[:], oat[:])
                nc.sync.dma_start(d["out_att"][m * 128:(m + 1) * 128, bi * BP:(bi + 1) * BP],
                                  oat16[:])

            xpb = [blk.tile([64, BP], F32, tag=f"xpb{j}", name=f"xpb{j}") for j in range(4)]
            for j in range(4):
                nc.sync.dma_start(xpb[j][:], xpT_d[j * 64:(j + 1) * 64,
                                                   bi * BP:(bi + 1) * BP])
            xpq = [blk1.tile([64, BP], F32R, tag=f"xpq{j}", name=f"xpq{j}") for j in range(4)]
            for j in range(4):
                nc.scalar.activation(xpq[j][:], xpb[j][:], AFT.Square)
            invl = smp.tile([8, BP], F32, tag="invl", name="invl")
            for n0 in range(0, BP, 512):
                nsz = min(512, BP - n0)
                ssqps = ps_raw.tile([8, 512], F32, tag="rawt", name="ssq_ps")
                for j in range(4):
                    nc.tensor.matmul(ssqps[:, 0:nsz], bd8r[j][:],
                                     xpq[j][:, n0:n0 + nsz], start=(j == 0), stop=(j == 3))
                nc.scalar.activation(invl[:, n0:n0 + nsz], ssqps[:, 0:nsz], AFT.Sqrt)
            nc.vector.tensor_scalar(invl[:], invl[:], float(NORM_EPS), None, ALU.max)
            nc.vector.reciprocal(invl[:], invl[:])
            nc.vector.tensor_scalar(invl[:], invl[:], ab[0:8, 0:1], None, ALU.mult)
            invP = smp.tile([128, 8 * NPC], F32, tag="invP", name="invP")
            for pc in range(NPC):
                sz = min(128, BP - pc * 128)
                pt = ps_tp.tile([128, 8], F32, tag="tp", name="inv_tp")
                nc.tensor.transpose(pt[0:sz, :], invl[:, pc * 128:pc * 128 + sz],
                                    ident[0:8, 0:8])
                nc.scalar.copy(invP[0:sz, pc * 8:(pc + 1) * 8], pt[0:sz, :])

            maskT = [mkp.tile([64, BP], F32R, tag=f"maskT{h}", name=f"maskT{h}")
                     for h in range(8)]
            for h in range(8):
                bp = (h % 2) * 32
                for pc in range(NPC):
                    sz = min(128, BP - pc * 128)
                    raw = ps_raw.tile([128, 64], F32, tag="rawt", name="raw2")
                    nc.tensor.matmul(
                        raw[0:sz, :],
                        xpb[h // 2][bp:bp + 32, pc * 128:pc * 128 + sz],
                        apnP[h][bp:bp + 32, :], start=True, stop=True)
                    mx = smp.tile([128, 1], F32, tag="mx2", name="mx2")
                    nc.vector.reduce_max(mx[0:sz, :], raw[0:sz, :], AXX)
                    vals = smp.tile([128, 1], F32, tag="vals2", name="vals2")
                    nc.scalar.activation(vals[0:sz, :], mx[0:sz, :], AFT.Sigmoid,
                                         bias=ab[0:sz, 1:2],
                                         scale=invP[0:sz, pc * 8 + h:pc * 8 + h + 1])
                    mkw = smp.tile([128, 64], F32, tag="mkw", name="mkw")
                    nc.vector.tensor_scalar(mkw[0:sz, :], raw[0:sz, :], mx[0:sz, :],
                                            vals[0:sz, :], ALU.is_equal, ALU.mult)
                    pt = ps_tp.tile([64, 128], F32, tag="tp", name="mask_tp")
                    nc.tensor.transpose(pt[:, 0:sz], mkw[0:sz, :], ident[0:sz, 0:sz])
                    nc.scalar.copy(maskT[h][:, pc * 128:pc * 128 + sz], pt[:, 0:sz])
            occ = [outp.tile([128, BP], F16, tag=f"occ{m}", name=f"occ{m}")
                   for m in range(2)]
            for n0 in range(0, BP, 512):
                nsz = min(512, BP - n0)
                dispS = []
                for h in range(8):
                    dps = ps_dsp.tile([32, 512], F32, tag="disp_ps", name="disp_ps")
                    nc.tensor.matmul(dps[:, 0:nsz], avR[h][:],
                                     maskT[h][:, n0:n0 + nsz], start=True, stop=True)
                    dsb = blk1.tile([32, 512], F32R, tag=f"dispS{h}", name=f"dispS{h}")
                    nc.scalar.copy(dsb[:, 0:nsz], dps[:, 0:nsz])
                    dispS.append(dsb)
                for m in range(2):
                    pst = ps1.tile([128, 512], F32, tag="big_ps", name="coc_ps")
                    for h in range(8):
                        nc.tensor.matmul(pst[:, 0:nsz],
                                         wm1R[h][:, m * 128:(m + 1) * 128],
                                         dispS[h][:, 0:nsz],
                                         start=(h == 0), stop=(h == 7))
                    nc.scalar.activation(occ[m][:, n0:n0 + nsz], pst[:, 0:nsz],
                                         AFT.Identity, bias=bm1[:, m:m + 1])
            for m in range(2):
                nc.sync.dma_start(
                    d["out_coc"][m * 128:(m + 1) * 128, bi * BP:(bi + 1) * BP], occ[m][:])


def build_fused():
    nc = bacc.Bacc("TRN2", num_devices=NCORE, debug=False)
    d = {}
    d["x"] = nc.dram_tensor("x", [C, PIX], F32, kind="ExternalInput").ap()
    d["wdq"] = nc.dram_tensor("wdq", [C, 16], F32, kind="ExternalInput").ap()
    d["bdq"] = nc.dram_tensor("bdq", [C, 1], F32, kind="ExternalInput").ap()
    d["lnw"] = nc.dram_tensor("lnw", [1, C], F32, kind="ExternalInput").ap()
    d["lnb"] = nc.dram_tensor("lnb", [1, C], F32, kind="ExternalInput").ap()
    d["wqk"] = nc.dram_tensor("wqk", [C, HID], F32, kind="ExternalInput").ap()
    d["bqk"] = nc.dram_tensor("bqk", [HID, 1], F32, kind="ExternalInput").ap()
    d["wv"] = nc.dram_tensor("wv", [C, HID], F32, kind="ExternalInput").ap()
    d["bv"] = nc.dram_tensor("bv", [HID, 1], F32, kind="ExternalInput").ap()
    d["wpt"] = nc.dram_tensor("wpt", [C, HID], F32, kind="ExternalInput").ap()
    d["bpt"] = nc.dram_tensor("bpt", [HID, 1], F32, kind="ExternalInput").ap()
    d["dab"] = nc.dram_tensor("dab", [1, 2], F32, kind="ExternalInput").ap()
    d["ab"] = nc.dram_tensor("ab", [1, 2], F32, kind="ExternalInput").ap()
    d["wm0"] = nc.dram_tensor("wm0", [HID, C], F32, kind="ExternalInput").ap()
    d["bm0"] = nc.dram_tensor("bm0", [C, 1], F32, kind="ExternalInput").ap()
    d["wm1"] = nc.dram_tensor("wm1", [HID, C], F32, kind="ExternalInput").ap()
    d["bm1"] = nc.dram_tensor("bm1", [C, 1], F32, kind="ExternalInput").ap()
    d["bd8"] = nc.dram_tensor("bd8", [C, 8], F32, kind="ExternalInput").ap()
    d["ident"] = nc.dram_tensor("ident", [128, 128], F32, kind="ExternalInput").ap()
    d["out_att"] = nc.dram_tensor("out_att", [C, PIX], F16, kind="ExternalOutput").ap()
    d["out_coc"] = nc.dram_tensor("out_coc", [C, PIX], F16, kind="ExternalOutput").ap()

    with tile.TileContext(nc) as tc:
        with tc.tile_pool(name="dram", bufs=1, space="DRAM") as dramp, \
             tc.tile_pool(name="gconst", bufs=1) as gcp:
            ident = gcp.tile([128, 128], F32, name="identg")
            nc.sync.dma_start(ident[:], d["ident"][:, :])

            xpT_d = dramp.tile([C, PIX], F32)
            a1i = dramp.tile([C, X1W], F32)
            a1o = dramp.tile([C, X1W], F32)
            a2i = dramp.tile([C, MW], F32)
            a2o = dramp.tile([C, MW], F32)
            a3i = dramp.tile([C, 128], F32)
            a3o = dramp.tile([C, 128], F32)

            _phase_L1(nc, tc, d, ident, xpT_d, a1i)
            nc.gpsimd.collective_compute(
                "AllToAll", ALU.bypass,
                replica_groups=[list(range(NCORE))],
                ins=[a1i.opt()], outs=[a1o.opt()])
            _phase_A(nc, tc, d, ident, a1o, a2i, a3i)
            nc.gpsimd.collective_compute(
                "AllToAll", ALU.bypass,
                replica_groups=[list(range(NCORE))],
                ins=[a2i.opt()], outs=[a2o.opt()])
            nc.gpsimd.collective_compute(
                "AllToAll", ALU.bypass,
                replica_groups=[list(range(NCORE))],
                ins=[a3i.opt()], outs=[a3o.opt()])
            _phase_B(nc, tc, d, ident, a2o, a3o, xpT_d)
    nc.compile()
    return nc
